# revision 11
# baseline (speedup 1.0000x reference)
"""Trainium2 Bass kernel for windowed/global sparse attention (Swin-style
relative-position bias + 1 global token), data-parallel over batch on 8 cores.

Shapes: B=16, N=785 (1 global + 28x28 local), C=768, H=12 heads, d=64.

Per-core device program (2 batches/core):
  - qT/kT computed transposed ([d, tokens]) so S^T = K @ Q^T needs no
    transposes anywhere; v computed natural ([tokens, d]) with a ones column
    appended per head so the P @ V matmul also yields softmax denominators.
  - softmax: exp(S + bias) = exp(S) * expB with expB = exp(bias) gathered on
    host from the (tiny) relative-position table at constant indices and
    shipped as a bf16 input; exp on ScalarE, multiply on VectorE (bf16 2x).
  - normalization: denominators from all 12 heads are staged to DRAM, one
    batched DVE reciprocal, then DMA-broadcast (0-step DRAM source) back to
    [128, N] and multiplied in place into O^T; proj consumes O^T directly as
    lhsT (again no transposes).
"""

import numpy as np
import ml_dtypes

import concourse.bass as bass
import concourse.bacc as bacc
import concourse.tile as tile
from concourse import mybir
from concourse.bass_utils import run_bass_kernel_spmd

F32 = mybir.dt.float32
F32R = mybir.dt.float32r
BF16 = mybir.dt.bfloat16

WX = WY = 28
NGLO = 1
H = 12
L = WX * WY            # 784
N = NGLO + L           # 785
C = 768
HD = C // H            # 64
SCALE = HD ** -0.5
B = 16
N_CORES = 8
B_LOC = B // N_CORES   # 2
NCC = C // 128         # 6 contraction chunks
NKC = (N + 127) // 128  # 7 key/token chunks (last = 17 rows)
W = 786                # padded free width for N-sized tiles (even, 4B-aligned)

# column groups (start, size) for N=785 and 768 free dims (<=512 per PSUM bank)
# N groups are padded to even sizes (fp32r matmul dest-pattern ISA restriction)
CG_N = [(0, 512), (512, 274)]
CG_C = [(0, 512), (512, 256)]


def _kr(kc):
    return min(128, N - kc * 128)


def build_nc():
    nc = bacc.Bacc(None, target_bir_lowering=False)

    xT_d = nc.dram_tensor("xT", [B_LOC, C, N], F32, kind="ExternalInput")
    qkvwT_d = nc.dram_tensor("qkv_wT", [C, 3 * C], F32, kind="ExternalInput")
    pwT_d = nc.dram_tensor("proj_wT", [C, C], F32, kind="ExternalInput")
    pb_d = nc.dram_tensor("proj_b", [1, C], F32, kind="ExternalInput")
    expB_d = nc.dram_tensor("expB", [H, N, N], BF16, kind="ExternalInput")
    out_d = nc.dram_tensor("out", [B_LOC, N, C], F32, kind="ExternalOutput")
    den_d = nc.dram_tensor("den_scratch", [B_LOC, H, N], F32)
    dinv_d = nc.dram_tensor("dinv_scratch", [B_LOC, H, N], F32)

    with tile.TileContext(nc) as tc:
        with (
            tc.tile_pool(name="consts", bufs=1) as consts,
            tc.tile_pool(name="stage", bufs=2) as stage,
            tc.tile_pool(name="perb", bufs=1) as perb,
            tc.tile_pool(name="expbp", bufs=2) as expbp,
            tc.tile_pool(name="flow", bufs=3) as flow,
            tc.tile_pool(name="norm", bufs=2) as norm,
            tc.tile_pool(name="outp", bufs=3) as outp,
            tc.tile_pool(name="psum", bufs=4, space=bass.MemorySpace.PSUM) as psum,
        ):
            # ---- weights (resident) ----
            qkvw = []
            for cc in range(NCC):
                t = consts.tile([128, 3 * C], F32R, tag=f"qkvw{cc}")
                nc.sync.dma_start(
                    t[:], qkvwT_d[cc * 128:(cc + 1) * 128, :].bitcast(F32R)
                )
                qkvw.append(t)
            pw16 = []
            for cc in range(NCC):
                st = stage.tile([128, C], F32, tag="pwstage")
                nc.sync.dma_start(st[:], pwT_d[cc * 128:(cc + 1) * 128, :])
                t = consts.tile([128, C], BF16, tag=f"pw{cc}")
                nc.vector.tensor_copy(t[:], st[:])
                pw16.append(t)
            pb_rep = consts.tile([128, C], F32, tag="pbrep")
            nc.sync.dma_start(pb_rep[:], pb_d[:].to_broadcast([128, C]))

            for b in range(B_LOC):
                # ---- load x^T ----
                xts = []
                for cc in range(NCC):
                    t = perb.tile([128, W], F32R, tag=f"xt{cc}")
                    nc.sync.dma_start(
                        t[:, 0:N],
                        xT_d[b, cc * 128:(cc + 1) * 128, :].bitcast(F32R),
                    )
                    nc.vector.memset(t[:, N:W].bitcast(F32), 0.0)
                    xts.append(t)

                # ---- QKV-T: q^T,k^T chunks [128, N] (chunk oc of 12) ----
                qT = [perb.tile([128, W], BF16, tag=f"qT{i}", name=f"qT{i}_{b}") for i in range(NCC)]
                kT = [perb.tile([128, W], BF16, tag=f"kT{i}", name=f"kT{i}_{b}") for i in range(NCC)]
                for oc in range(2 * NCC):
                    ps = psum.tile([128, W], F32, tag="ps")
                    for cc in range(NCC):
                        for (c0, cn) in CG_N:
                            nc.tensor.matmul(
                                ps[:, c0:c0 + cn],
                                qkvw[cc][:, oc * 128:(oc + 1) * 128],
                                xts[cc][:, c0:c0 + cn],
                                start=(cc == 0),
                                stop=(cc == NCC - 1),
                            )
                    dst = qT[oc] if oc < NCC else kT[oc - NCC]
                    nc.scalar.copy(dst[:, 0:N], ps[:, 0:N])
                    nc.vector.memset(dst[:, N:W], 0.0)

                # ---- V natural with ones column: vp[kc] = [128, H*(HD+1)] ----
                vp = [perb.tile([128, H * (HD + 1)], BF16, tag=f"vp{i}", name=f"vp{i}_{b}")
                      for i in range(NKC)]
                for kc in range(NKC):
                    kr = _kr(kc)
                    ps = psum.tile([128, C], F32, tag="ps")
                    for cc in range(NCC):
                        for (c0, cn) in CG_C:
                            nc.tensor.matmul(
                                ps[0:kr, c0:c0 + cn],
                                xts[cc][:, kc * 128:kc * 128 + kr],
                                qkvw[cc][:, 2 * C + c0:2 * C + c0 + cn],
                                start=(cc == 0),
                                stop=(cc == NCC - 1),
                            )
                    v3 = vp[kc][:].rearrange("p (h e) -> p h e", e=HD + 1)
                    nc.vector.tensor_copy(
                        v3[0:kr, :, 0:HD],
                        ps[0:kr, :].rearrange("p (h d) -> p h d", d=HD),
                    )
                    nc.vector.memset(v3[0:kr, :, HD:HD + 1], 1.0)

                # ---- attention per head ----
                oT = [perb.tile([128, W], BF16, tag=f"oT{i}", name=f"oT{i}_{b}") for i in range(NCC)]
                for h in range(H):
                    cch, po = h // 2, (h % 2) * 64
                    ebt = expbp.tile([128, NKC * W], BF16, tag="expb")
                    nc.vector.memset(
                        ebt[:].rearrange("p (k w) -> p k w", w=W)[:, :, N:W], 0.0
                    )
                    for kc in range(NKC):
                        kr = _kr(kc)
                        nc.sync.dma_start(
                            ebt[0:kr, kc * W:kc * W + N],
                            expB_d[h, kc * 128:kc * 128 + kr, :],
                        )
                    ps_o = psum.tile([HD + 1, W], F32, tag="ps")
                    for kc in range(NKC):
                        kr = _kr(kc)
                        ps_s = psum.tile([128, W], F32, tag="ps")
                        for (c0, cn) in CG_N:
                            nc.tensor.matmul(
                                ps_s[0:kr, c0:c0 + cn],
                                kT[cch][po:po + 64, kc * 128:kc * 128 + kr],
                                qT[cch][po:po + 64, c0:c0 + cn],
                                start=True,
                                stop=True,
                            )
                        es = flow.tile([128, W], BF16, tag="expS")
                        nc.scalar.activation(
                            es[0:kr, 0:W], ps_s[0:kr, 0:W],
                            mybir.ActivationFunctionType.Exp,
                        )
                        pt = flow.tile([128, W], BF16, tag="pT")
                        nc.vector.tensor_tensor(
                            pt[0:kr, 0:W],
                            es[0:kr, 0:W],
                            ebt[0:kr, kc * W:(kc + 1) * W],
                            mybir.AluOpType.mult,
                        )
                        for (c0, cn) in CG_N:
                            nc.tensor.matmul(
                                ps_o[:, c0:c0 + cn],
                                vp[kc][0:kr, h * (HD + 1):(h + 1) * (HD + 1)],
                                pt[0:kr, c0:c0 + cn],
                                start=(kc == 0),
                                stop=(kc == NKC - 1),
                            )
                    # evacuate unnormalized O^T + denominator row to DRAM
                    nc.scalar.copy(oT[cch][po:po + 64, 0:N], ps_o[0:64, 0:N])
                    dn = norm.tile([65, W], F32, tag="dn")
                    nc.vector.tensor_copy(dn[64:65, 0:N], ps_o[64:65, 0:N])
                    nc.sync.dma_start(den_d[b, h, :], dn[64:65, 0:N])

                # ---- batched reciprocal + broadcast + in-place normalize ----
                dall = norm.tile([12, W], F32, tag="dall")
                nc.sync.dma_start(dall[0:H, 0:N], den_d[b])
                dinv = norm.tile([12, W], F32, tag="dinv")
                nc.vector.reciprocal(dinv[0:H, 0:N], dall[0:H, 0:N])
                nc.sync.dma_start(dinv_d[b], dinv[0:H, 0:N])
                for cc in range(NCC):
                    dr = norm.tile([128, W], F32, tag="drep")
                    for hh in range(2):
                        row = dinv_d[b, 2 * cc + hh, :]
                        src = bass.AP(
                            tensor=row.tensor, offset=row.offset,
                            ap=[[0, 64]] + row.ap,
                        )
                        nc.sync.dma_start(dr[hh * 64:(hh + 1) * 64, 0:N], src)
                    nc.vector.tensor_tensor(
                        oT[cc][:, 0:N], oT[cc][:, 0:N], dr[:, 0:N],
                        mybir.AluOpType.mult,
                    )

                # ---- proj + bias ----
                for tt in range(NKC):
                    ts_ = _kr(tt)
                    ps = psum.tile([128, C], F32, tag="ps")
                    for cc in range(NCC):
                        for (c0, cn) in CG_C:
                            nc.tensor.matmul(
                                ps[0:ts_, c0:c0 + cn],
                                oT[cc][:, tt * 128:tt * 128 + ts_],
                                pw16[cc][:, c0:c0 + cn],
                                start=(cc == 0),
                                stop=(cc == NCC - 1),
                            )
                    ob = outp.tile([128, C], F32, tag="ob")
                    nc.vector.tensor_tensor(
                        ob[0:ts_, :], ps[0:ts_, :], pb_rep[0:ts_, :],
                        mybir.AluOpType.add,
                    )
                    nc.sync.dma_start(
                        out_d[b, tt * 128:tt * 128 + ts_, :], ob[0:ts_, :]
                    )

    nc.compile()
    return nc


def _relative_position_index():
    coords = np.stack(np.meshgrid(np.arange(WX), np.arange(WY), indexing="ij"))
    cf = coords.reshape(2, -1)
    rel = cf[:, :, None] - cf[:, None, :]
    rel = rel.transpose(1, 2, 0).astype(np.int64)
    rel[:, :, 0] += WX - 1
    rel[:, :, 1] += WY - 1
    rel[:, :, 0] *= 2 * WY - 1
    return rel.sum(-1)  # [L, L]


def _host_prep(x, qkv_w, proj_w, proj_b, rel_table, g2l, g2g):
    x = np.asarray(x, np.float32)
    qkv_w = np.asarray(qkv_w, np.float32)
    proj_w = np.asarray(proj_w, np.float32)
    proj_b = np.asarray(proj_b, np.float32)
    rel_table = np.asarray(rel_table, np.float32)
    g2l = np.asarray(g2l, np.float32)
    g2g = np.asarray(g2g, np.float32)

    xT = np.ascontiguousarray(x.transpose(0, 2, 1))          # [B, C, N]
    qkv_wT = np.ascontiguousarray(qkv_w.T).copy()            # [C, 3C]
    qkv_wT[:, :C] *= SCALE                                   # fold q scale
    proj_wT = np.ascontiguousarray(proj_w.T)                 # [C, C]
    pb = proj_b.reshape(1, C)

    # expB[h, k, q] = exp(bias[h, q, k]); exp applied at table granularity,
    # then expanded by the constant-index relative-position gather.
    ridx = _relative_position_index()
    et = np.exp(rel_table)                                   # [3025, H]
    eg2l = np.exp(g2l)                                       # [2, H, 1]
    eg2g = np.exp(g2g)                                       # [H, 1, 1]
    expB = np.empty((H, N, N), np.float32)
    # local block: bias[h, q, k] = rel_table[ridx[q-1, k-1], h]
    expB[:, 1:, 1:] = et[ridx].transpose(2, 1, 0)            # [H, k, q]
    expB[:, 0, 0] = eg2g[:, 0, 0]
    expB[:, 1:, 0] = eg2l[0][:, 0][None, :].T                # global query row
    expB[:, 0, 1:] = eg2l[1][:, 0][:, None]                  # global key col
    expB16 = expB.astype(ml_dtypes.bfloat16)

    in_maps = []
    for i in range(N_CORES):
        in_maps.append({
            "xT": xT[i * B_LOC:(i + 1) * B_LOC],
            "qkv_wT": qkv_wT,
            "proj_wT": proj_wT,
            "proj_b": pb,
            "expB": expB16,
        })
    return in_maps


_NC = None


def get_nc():
    global _NC
    if _NC is None:
        _NC = build_nc()
    return _NC


def kernel(x, qkv_w, proj_w, proj_b, rel_table, g2l, g2g):
    in_maps = _host_prep(x, qkv_w, proj_w, proj_b, rel_table, g2l, g2g)
    nc = get_nc()
    res = run_bass_kernel_spmd(nc, in_maps, core_ids=list(range(N_CORES)))
    out = np.concatenate([res.results[i]["out"] for i in range(N_CORES)], axis=0)
    return out.astype(np.float32)
